# revision 3
# baseline (speedup 1.0000x reference)
"""MoE top-1 routing kernel for Trainium2 (8 NeuronCores).

Problem: x [N=8192, D=2048] f32, indices [N,1] int (expert id in [0,8)),
W [E=8, D, H=2048] f32, b [E, H] f32.
Output: tokens sorted (stably) by expert id, each row = relu(x @ W[e] + b[e]).

Sharding: experts are paired (hot with cold, to balance token counts) and
each pair of cores splits the output dim H in half.  Core 2i computes
h[0:1024] and core 2i+1 computes h[1024:2048] for both experts of pair i.
The host routes tokens (stable argsort by expert id == the required output
order) and ships x^T segments; the device computes
y^T = relu(W^T @ x^T + b) with W stationary in SBUF.

The k (contraction) loop is outermost within each token chunk so the PE
only needs one 0.5 MB W tile to start, letting the W DMA stream hide
behind compute.  Section sizes CA (first expert) and CB (second expert)
are uniform across cores so the single SPMD instruction stream is valid
everywhere (per-core variation lives purely in the input data).
"""

import math

import numpy as np

import concourse.bass as bass
import concourse.mybir as mybir
import concourse.tile as tile
from concourse import bacc
from concourse.bass_utils import run_bass_kernel_spmd

P = 128           # SBUF partitions
D = 2048          # input features (contraction dim)
H = 2048          # output features
HH = H // 2       # per-core output slice
E = 8             # experts
NT = 256          # token chunk (matmul moving free dim; >=256 keeps f32r at 1 cyc/row)
KT = D // P       # 16 contraction chunks
MT = HH // P      # 8 output-partition chunks per core

_PROGRAM_CACHE: dict = {}


def _build_program(CA: int, CB: int) -> bass.Bass:
    """One-core SPMD program: yT[HH, CA+CB] = relu(Wc[s]^T @ xT + bc[s]).

    Token columns [0, CA) belong to expert slot 0, [CA, CA+CB) to slot 1.
    """
    assert CA % NT == 0 and CB % NT == 0
    C2 = CA + CB
    NCH = C2 // NT

    nc = bacc.Bacc(None, target_bir_lowering=False, debug=False)

    xT = nc.dram_tensor("xT", [D, C2], mybir.dt.float32r, kind="ExternalInput")
    Wc = nc.dram_tensor("Wc", [2, D, HH], mybir.dt.float32r, kind="ExternalInput")
    bc = nc.dram_tensor("bc", [2 * MT, P], mybir.dt.float32, kind="ExternalInput")
    yT = nc.dram_tensor("yT", [HH, C2], mybir.dt.float32, kind="ExternalOutput")

    with tile.TileContext(nc) as tc:
        with (
            tc.tile_pool(name="wpool", bufs=1) as wpool,
            tc.tile_pool(name="xpool", bufs=2) as xpool,
            tc.tile_pool(name="opool", bufs=4) as opool,
            tc.tile_pool(name="bpool", bufs=1) as bpool,
            tc.tile_pool(name="pspool", bufs=8, space="PSUM") as pspool,
        ):
            btile = bpool.tile([P, 2 * MT], mybir.dt.float32, name="btile")
            nc.sync.dma_start(btile[:], bc[:].rearrange("m p -> p m"))

            # W resident in SBUF: 32 tiles of [128, HH]; slot-0 k tiles first
            # so the first token chunk's weights arrive first.
            wt = []
            for s in range(2):
                for k in range(KT):
                    wk = wpool.tile([P, HH], mybir.dt.float32r,
                                    name=f"w{s}_{k}", tag=f"w{s}_{k}")
                    nc.sync.dma_start(wk[:], Wc[s, k * P:(k + 1) * P, :])
                    wt.append(wk)

            for n in range(NCH):
                sel = 0 if n * NT < CA else 1
                xt = []
                for k in range(KT):
                    xk = xpool.tile([P, NT], mybir.dt.float32r,
                                    name=f"x{k}", tag=f"x{k}")
                    nc.sync.dma_start(
                        xk[:], xT[k * P:(k + 1) * P, n * NT:(n + 1) * NT])
                    xt.append(xk)
                ps = []
                for m in range(MT):
                    pm = pspool.tile([P, NT], mybir.dt.float32,
                                     name=f"ps{m}", tag="ps")
                    ps.append(pm)
                for k in range(KT):
                    wk = wt[sel * KT + k]
                    for m in range(MT):
                        nc.tensor.matmul(
                            ps[m][:],
                            wk[:, m * P:(m + 1) * P],   # stationary [K=128, M=128]
                            xt[k][:],                   # moving     [K=128, NT]
                            start=(k == 0),
                            stop=(k == KT - 1),
                        )
                for m in range(MT):
                    ot = opool.tile([P, NT], mybir.dt.float32, name="ot", tag="ot")
                    nc.scalar.activation(
                        ot[:], ps[m][:],
                        mybir.ActivationFunctionType.Relu,
                        bias=btile[:, sel * MT + m:sel * MT + m + 1],
                    )
                    nc.sync.dma_start(
                        yT[m * P:(m + 1) * P, n * NT:(n + 1) * NT], ot[:])
    nc.compile()
    return nc


def _get_program(CA: int, CB: int) -> bass.Bass:
    key = (CA, CB)
    if key not in _PROGRAM_CACHE:
        _PROGRAM_CACHE[key] = _build_program(CA, CB)
    return _PROGRAM_CACHE[key]


def _pad(n: int) -> int:
    return int(max(NT, math.ceil(n / NT) * NT))


def _route(x, indices):
    """Host-side routing: stable sort by expert, hot/cold pairing, padding."""
    idx = np.asarray(indices).reshape(-1).astype(np.int64)
    order = np.argsort(idx, kind="stable")
    counts = np.bincount(idx, minlength=E)
    starts = np.concatenate([[0], np.cumsum(counts)])
    tok = {e: order[starts[e]:starts[e + 1]] for e in range(E)}

    by_count = np.argsort(-counts, kind="stable")
    pairs = [(int(by_count[i]), int(by_count[E - 1 - i])) for i in range(E // 2)]
    CA = _pad(max(int(counts[a]) for a, _ in pairs))
    CB = _pad(max(int(counts[b]) for _, b in pairs))
    return order, counts, tok, pairs, CA, CB


def _build_in_maps(x, W, b, counts, tok, pairs, CA, CB):
    x = np.asarray(x, dtype=np.float32)
    W = np.asarray(W, dtype=np.float32)
    b = np.asarray(b, dtype=np.float32)
    in_maps = []
    for (ea, eb) in pairs:
        xT_pair = np.zeros((D, CA + CB), dtype=np.float32)
        ca, cb = int(counts[ea]), int(counts[eb])
        if ca:
            xT_pair[:, :ca] = x[tok[ea]].T
        if cb:
            xT_pair[:, CA:CA + cb] = x[tok[eb]].T
        for half in range(2):
            hs = slice(half * HH, (half + 1) * HH)
            in_maps.append({
                "xT": xT_pair,
                "Wc": np.ascontiguousarray(
                    np.stack([W[ea][:, hs], W[eb][:, hs]])),
                "bc": np.ascontiguousarray(
                    np.concatenate([b[ea][hs], b[eb][hs]]).reshape(2 * MT, P)),
            })
    return in_maps


def _assemble(results, N, counts, pairs, CA):
    out = np.empty((N, H), dtype=np.float32)
    starts = {}
    pos = 0
    for e in range(E):
        starts[e] = pos
        pos += int(counts[e])
    for i, (ea, eb) in enumerate(pairs):
        ca, cb = int(counts[ea]), int(counts[eb])
        for half in range(2):
            yT = results[2 * i + half]["yT"]
            hs = slice(half * HH, (half + 1) * HH)
            if ca:
                out[starts[ea]:starts[ea] + ca, hs] = yT[:, :ca].T
            if cb:
                out[starts[eb]:starts[eb] + cb, hs] = yT[:, CA:CA + cb].T
    return out


def kernel(x, indices, W, b):
    x = np.asarray(x, dtype=np.float32)
    N = x.shape[0]
    order, counts, tok, pairs, CA, CB = _route(x, indices)
    nc = _get_program(CA, CB)
    in_maps = _build_in_maps(x, W, b, counts, tok, pairs, CA, CB)
    results = run_bass_kernel_spmd(nc, in_maps, list(range(E))).results
    return _assemble(results, N, counts, pairs, CA)


# revision 4
# speedup vs baseline: 1.2242x; 1.2242x over previous
"""MoE top-1 routing kernel for Trainium2 (8 NeuronCores).

Problem: x [N=8192, D=2048] f32, indices [N,1] int (expert id in [0,8)),
W [E=8, D, H=2048] f32, b [E, H] f32.
Output: tokens sorted (stably) by expert id, each row = relu(x @ W[e] + b[e]).

Sharding: experts are paired (hot with cold, to balance token counts) and
each pair of cores splits the output dim H in half.  Core 2i computes
h[0:1024] and core 2i+1 computes h[1024:2048] for both experts of pair i.
The host routes tokens (stable argsort by expert id == the required output
order) and ships transposed/swizzled segments; the device computes
y^T = relu(W^T @ x^T + b) with W stationary in SBUF.

Device program structure (per core, SPMD):
  - W [2 experts x 2048 x 1024] lives in SBUF as the matmul stationary
    operand, streamed in 2.1 MB DMAs on the scalar HWDGE ring.
  - Tokens are processed in 256-wide chunks; each chunk's x^T arrives as a
    single 2 MB DMA (host pre-swizzled so each SBUF partition reads one
    contiguous 16 KB run) on the sync HWDGE ring.
  - Within a chunk the contraction (k) loop is outermost and all 8 PSUM
    banks hold the chunk's 8 output-row tiles, so the PE needs only one W
    k-tile to start and the W stream hides behind compute.
  - PSUM eviction fuses bias + ReLU on the scalar engine; a chunk's 8
    output tiles are collected in one SBUF tile and leave as a single
    1 MB DMA (host un-swizzles).
  - Section sizes CA/CB (tokens of first/second expert, padded to 256) are
    uniform across cores so one SPMD instruction stream serves all cores;
    per-core variation lives purely in the input data.

Matmuls run in float32r (full fp32 storage; ~1 PE cycle/row for moving
free dim >= 256, vs 4 for plain float32).
"""

import math

import numpy as np

import concourse.bass as bass
import concourse.mybir as mybir
import concourse.tile as tile
from concourse import bacc
from concourse.bass_utils import run_bass_kernel_spmd

P = 128           # SBUF partitions
D = 2048          # input features (contraction dim)
H = 2048          # output features
HH = H // 2       # per-core output slice
E = 8             # experts
NT = 256          # token chunk (matmul moving free dim; >=256 keeps f32r at 1 cyc/row)
KT = D // P       # 16 contraction chunks
MT = HH // P      # 8 output-partition chunks per core
KG = 4            # W k-tiles per DMA (2.1 MB transfers)

_PROGRAM_CACHE: dict = {}


def _build_program(CA: int, CB: int) -> bass.Bass:
    """One-core SPMD program over token sections [0,CA) -> slot 0, [CA,CA+CB) -> slot 1."""
    assert CA % NT == 0 and CB % NT == 0
    C2 = CA + CB
    NCH = C2 // NT

    nc = bacc.Bacc(None, target_bir_lowering=False, debug=False)

    # Host-swizzled layouts (see _build_in_maps):
    #   xs[n, p, k*NT + t] = x^T[k*P + p, n*NT + t]
    #   Wc[s, p, k*HH + h] = W[expert_s][k*P + p, half*HH + h]
    #   ys[n, p, m*NT + t] = y^T[m*P + p, n*NT + t]
    xs = nc.dram_tensor("xs", [NCH, P, KT * NT], mybir.dt.float32r,
                        kind="ExternalInput")
    Wc = nc.dram_tensor("Wc", [2, P, KT * HH], mybir.dt.float32r,
                        kind="ExternalInput")
    bc = nc.dram_tensor("bc", [P, 2 * MT], mybir.dt.float32, kind="ExternalInput")
    ys = nc.dram_tensor("ys", [NCH, P, MT * NT], mybir.dt.float32,
                        kind="ExternalOutput")

    with tile.TileContext(nc) as tc:
        with (
            tc.tile_pool(name="wpool", bufs=1) as wpool,
            tc.tile_pool(name="xpool", bufs=2) as xpool,
            tc.tile_pool(name="opool", bufs=2) as opool,
            tc.tile_pool(name="bpool", bufs=1) as bpool,
            tc.tile_pool(name="pspool", bufs=8, space="PSUM") as pspool,
        ):
            btile = bpool.tile([P, 2 * MT], mybir.dt.float32, name="btile")
            nc.sync.dma_start(btile[:], bc[:])

            # W resident in SBUF; KG k-tiles per DMA, slot 0 first so the
            # first token chunk's weights arrive first.  Scalar HWDGE ring
            # keeps the W stream off the x/out ring.
            wt = {}
            for s in range(2):
                for g in range(KT // KG):
                    wg = wpool.tile([P, KG * HH], mybir.dt.float32r,
                                    name=f"w{s}_{g}", tag=f"w{s}_{g}")
                    nc.scalar.dma_start(
                        wg[:], Wc[s, :, g * KG * HH:(g + 1) * KG * HH])
                    wt[(s, g)] = wg

            for n in range(NCH):
                sel = 0 if n * NT < CA else 1
                xt = xpool.tile([P, KT * NT], mybir.dt.float32r,
                                name="xt", tag="xt")
                nc.sync.dma_start(xt[:], xs[n])
                ps = []
                for m in range(MT):
                    pm = pspool.tile([P, NT], mybir.dt.float32,
                                     name=f"ps{m}", tag="ps")
                    ps.append(pm)
                for k in range(KT):
                    wg = wt[(sel, k // KG)]
                    woff = (k % KG) * HH
                    for m in range(MT):
                        nc.tensor.matmul(
                            ps[m][:],
                            wg[:, woff + m * P:woff + (m + 1) * P],  # [K=128, M=128]
                            xt[:, k * NT:(k + 1) * NT],              # [K=128, NT]
                            start=(k == 0),
                            stop=(k == KT - 1),
                        )
                osup = opool.tile([P, MT * NT], mybir.dt.float32,
                                  name="osup", tag="osup")
                for m in range(MT):
                    nc.scalar.activation(
                        osup[:, m * NT:(m + 1) * NT], ps[m][:],
                        mybir.ActivationFunctionType.Relu,
                        bias=btile[:, sel * MT + m:sel * MT + m + 1],
                    )
                nc.sync.dma_start(ys[n], osup[:])
    nc.compile()
    return nc


def _get_program(CA: int, CB: int) -> bass.Bass:
    key = (CA, CB)
    if key not in _PROGRAM_CACHE:
        _PROGRAM_CACHE[key] = _build_program(CA, CB)
    return _PROGRAM_CACHE[key]


def _pad(n: int) -> int:
    return int(max(NT, math.ceil(n / NT) * NT))


def _route(x, indices):
    """Host-side routing: stable sort by expert, hot/cold pairing, padding."""
    idx = np.asarray(indices).reshape(-1).astype(np.int64)
    order = np.argsort(idx, kind="stable")
    counts = np.bincount(idx, minlength=E)
    starts = np.concatenate([[0], np.cumsum(counts)])
    tok = {e: order[starts[e]:starts[e + 1]] for e in range(E)}

    by_count = np.argsort(-counts, kind="stable")
    pairs = [(int(by_count[i]), int(by_count[E - 1 - i])) for i in range(E // 2)]
    CA = _pad(max(int(counts[a]) for a, _ in pairs))
    CB = _pad(max(int(counts[b]) for _, b in pairs))
    return order, counts, tok, pairs, CA, CB


def _swizzle_x(x, tok_a, tok_b, CA, CB):
    """[C2, D] padded token matrix -> [NCH, P, KT*NT] per-chunk-contiguous."""
    C2 = CA + CB
    xp = np.zeros((C2, D), dtype=np.float32)
    if len(tok_a):
        xp[:len(tok_a)] = x[tok_a]
    if len(tok_b):
        xp[CA:CA + len(tok_b)] = x[tok_b]
    # xs[n, p, k, t] = xp[n*NT + t, k*P + p]
    return np.ascontiguousarray(
        xp.reshape(C2 // NT, NT, KT, P).transpose(0, 3, 2, 1)
    ).reshape(C2 // NT, P, KT * NT)


def _swizzle_w(We, half):
    """W[e] [D, H] -> [P, KT*HH] for one H-half: Wc[p, k*HH+h] = W[k*P+p, hs+h]."""
    hs = slice(half * HH, (half + 1) * HH)
    return np.ascontiguousarray(
        We[:, hs].reshape(KT, P, HH).transpose(1, 0, 2)).reshape(P, KT * HH)


def _build_in_maps(x, W, b, counts, tok, pairs, CA, CB):
    x = np.asarray(x, dtype=np.float32)
    W = np.asarray(W, dtype=np.float32)
    b = np.asarray(b, dtype=np.float32)
    in_maps = []
    for (ea, eb) in pairs:
        xs_pair = _swizzle_x(x, tok[ea], tok[eb], CA, CB)
        for half in range(2):
            hs = slice(half * HH, (half + 1) * HH)
            bc = np.stack([b[ea][hs].reshape(MT, P),
                           b[eb][hs].reshape(MT, P)])  # [2, MT, P]
            in_maps.append({
                "xs": xs_pair,
                "Wc": np.stack([_swizzle_w(W[ea], half),
                                _swizzle_w(W[eb], half)]),
                "bc": np.ascontiguousarray(
                    bc.reshape(2 * MT, P).T),          # [P, 2*MT]
            })
    return in_maps


def _assemble(results, N, counts, pairs, CA, CB):
    out = np.empty((N, H), dtype=np.float32)
    starts = {}
    pos = 0
    for e in range(E):
        starts[e] = pos
        pos += int(counts[e])
    C2 = CA + CB
    for i, (ea, eb) in enumerate(pairs):
        ca, cb = int(counts[ea]), int(counts[eb])
        for half in range(2):
            ysw = results[2 * i + half]["ys"]  # [NCH, P, MT*NT]
            # y[tok n*NT+t, hs + m*P + p] = ysw[n, p, m*NT + t]
            y = ysw.reshape(C2 // NT, P, MT, NT).transpose(0, 3, 2, 1) \
                   .reshape(C2, HH)
            hs = slice(half * HH, (half + 1) * HH)
            if ca:
                out[starts[ea]:starts[ea] + ca, hs] = y[:ca]
            if cb:
                out[starts[eb]:starts[eb] + cb, hs] = y[CA:CA + cb]
    return out


def kernel(x, indices, W, b):
    x = np.asarray(x, dtype=np.float32)
    N = x.shape[0]
    order, counts, tok, pairs, CA, CB = _route(x, indices)
    nc = _get_program(CA, CB)
    in_maps = _build_in_maps(x, W, b, counts, tok, pairs, CA, CB)
    results = run_bass_kernel_spmd(nc, in_maps, list(range(E))).results
    return _assemble(results, N, counts, pairs, CA, CB)


# revision 6
# speedup vs baseline: 1.2441x; 1.0162x over previous
"""MoE top-1 routing kernel for Trainium2 (8 NeuronCores).

Problem: x [N=8192, D=2048] f32, indices [N,1] int (expert id in [0,8)),
W [E=8, D, H=2048] f32, b [E, H] f32.
Output: tokens sorted (stably) by expert id, each row = relu(x @ W[e] + b[e]).

Sharding: experts are paired (hot with cold, to balance token counts) and
each pair of cores splits the output dim H in half.  Core 2i computes
h[0:1024] and core 2i+1 computes h[1024:2048] for both experts of pair i.
The host routes tokens (stable argsort by expert id == the required output
order) and ships transposed/swizzled segments; the device computes
y^T = relu(W^T @ x^T + b) with W stationary in SBUF.

Device program structure (per core, SPMD):
  - W [2 experts x 2048 x 1024] lives in SBUF as the matmul stationary
    operand, streamed in 2.1 MB DMAs on the scalar HWDGE ring.
  - Tokens are processed in 256-wide chunks; each chunk's x^T arrives as a
    single 2 MB DMA (host pre-swizzled so each SBUF partition reads one
    contiguous 16 KB run) on the sync HWDGE ring.
  - Within a chunk the contraction (k) loop is outermost and all 8 PSUM
    banks hold the chunk's 8 output-row tiles, so the PE needs only one W
    k-tile to start and the W stream hides behind compute.
  - PSUM eviction fuses bias + ReLU on the scalar engine; a chunk's 8
    output tiles are collected in one SBUF tile and leave as a single
    1 MB DMA (host un-swizzles).
  - Section sizes CA/CB (tokens of first/second expert, padded to 256) are
    uniform across cores so one SPMD instruction stream serves all cores;
    per-core variation lives purely in the input data.

Matmuls run in float32r (full fp32 storage; ~1 PE cycle/row for moving
free dim >= 256, vs 4 for plain float32).
"""

import math

import numpy as np

import concourse.bass as bass
import concourse.mybir as mybir
import concourse.tile as tile
from concourse import bacc
from concourse.bass_utils import run_bass_kernel_spmd

P = 128           # SBUF partitions
D = 2048          # input features (contraction dim)
H = 2048          # output features
HH = H // 2       # per-core output slice
E = 8             # experts
NT = 256          # token chunk (matmul moving free dim; >=256 keeps f32r at 1 cyc/row)
KT = D // P       # 16 contraction chunks
MT = HH // P      # 8 output-partition chunks per core
KG = 4            # W k-tiles per DMA (2.1 MB transfers)

_PROGRAM_CACHE: dict = {}


def _build_program(CA: int, CB: int) -> bass.Bass:
    """One-core SPMD program over token sections [0,CA) -> slot 0, [CA,CA+CB) -> slot 1."""
    assert CA % NT == 0 and CB % NT == 0
    C2 = CA + CB
    NCH = C2 // NT

    nc = bacc.Bacc(None, target_bir_lowering=False, debug=False)

    # Host-swizzled layouts (see _build_in_maps):
    #   xs[n, p, k*NT + t] = x^T[k*P + p, n*NT + t]
    #   Wc[s, p, k*HH + h] = W[expert_s][k*P + p, half*HH + h]
    #   ys[n, p, m*NT + t] = y^T[m*P + p, n*NT + t]
    xs = nc.dram_tensor("xs", [NCH, P, KT * NT], mybir.dt.float32r,
                        kind="ExternalInput")
    Wc = nc.dram_tensor("Wc", [2, P, KT * HH], mybir.dt.float32r,
                        kind="ExternalInput")
    bc = nc.dram_tensor("bc", [P, 2 * MT], mybir.dt.float32, kind="ExternalInput")
    ys = nc.dram_tensor("ys", [NCH, P, MT * NT], mybir.dt.float32,
                        kind="ExternalOutput")

    # Chunks 0 and 1 are interleaved at W-tile granularity during the W
    # stream-in so the PE has ~2x work per arriving W byte and never
    # starves.  Requires two section-0 chunks.
    import os
    phase1 = CA >= 2 * NT and NCH >= 2 and not os.environ.get("NO_PHASE1")

    with tile.TileContext(nc) as tc:
        with (
            tc.tile_pool(name="wpool", bufs=1) as wpool,
            tc.tile_pool(name="xpool", bufs=3) as xpool,
            tc.tile_pool(name="opool", bufs=2) as opool,
            tc.tile_pool(name="bpool", bufs=1) as bpool,
            tc.tile_pool(name="pspool", bufs=8, space="PSUM") as pspool,
        ):
            btile = bpool.tile([P, 2 * MT], mybir.dt.float32, name="btile")
            nc.sync.dma_start(btile[:], bc[:])

            # Early x: chunks 0/1 split into KG-sized sub-DMAs, interleaved,
            # so the first k tiles land quickly (sync HWDGE ring).
            nearly = min(2, NCH) if phase1 else 1
            xearly = []
            for c in range(nearly):
                xc = xpool.tile([P, KT * NT], mybir.dt.float32r,
                                name="xt", tag="xt")
                xearly.append(xc)
            for g in range(KT // KG):
                lo, hi = g * KG * NT, (g + 1) * KG * NT
                for c in range(nearly):
                    nc.sync.dma_start(xearly[c][:, lo:hi], xs[c, :, lo:hi])

            # W resident in SBUF on the scalar HWDGE ring (keeps the 16.8 MB
            # stream off the x/out ring).  Slot 0's first group is split
            # per-k so the PE can start after ~0.5 MB.
            wtk = []
            for k in range(KG):
                wk = wpool.tile([P, HH], mybir.dt.float32r,
                                name=f"wk{k}", tag=f"wk{k}")
                nc.scalar.dma_start(wk[:], Wc[0, :, k * HH:(k + 1) * HH])
                wtk.append(wk)
            wt = {}
            for s in range(2):
                for g in range(KT // KG):
                    if s == 0 and g == 0:
                        continue
                    wg = wpool.tile([P, KG * HH], mybir.dt.float32r,
                                    name=f"w{s}_{g}", tag=f"w{s}_{g}")
                    nc.scalar.dma_start(
                        wg[:], Wc[s, :, g * KG * HH:(g + 1) * KG * HH])
                    wt[(s, g)] = wg

            def wap(s, k, m):
                if s == 0 and k < KG:
                    return wtk[k][:, m * P:(m + 1) * P]
                g, r = divmod(k, KG)
                return wt[(s, g)][:, r * HH + m * P:r * HH + (m + 1) * P]

            if phase1:
                # Chunks 0+1 share PSUM banks: one [P, 2*NT] bank tile per m,
                # halves hold chunk 0 / chunk 1.
                psb = [pspool.tile([P, 2 * NT], mybir.dt.float32,
                                   name=f"psb{m}", tag="ps") for m in range(MT)]
                for k in range(KT):
                    for m in range(MT):
                        for c in range(2):
                            nc.tensor.matmul(
                                psb[m][:, c * NT:(c + 1) * NT],
                                wap(0, k, m),
                                xearly[c][:, k * NT:(k + 1) * NT],
                                start=(k == 0),
                                stop=(k == KT - 1),
                            )
                for c in range(2):
                    osup = opool.tile([P, MT * NT], mybir.dt.float32,
                                      name="osup", tag="osup")
                    for m in range(MT):
                        nc.scalar.activation(
                            osup[:, m * NT:(m + 1) * NT],
                            psb[m][:, c * NT:(c + 1) * NT],
                            mybir.ActivationFunctionType.Relu,
                            bias=btile[:, m:m + 1],
                        )
                    nc.sync.dma_start(ys[c], osup[:])

            for n in range(2 if phase1 else 0, NCH):
                sel = 0 if n * NT < CA else 1
                if not phase1 and n < nearly:
                    xt = xearly[n]
                else:
                    xt = xpool.tile([P, KT * NT], mybir.dt.float32r,
                                    name="xt", tag="xt")
                    nc.sync.dma_start(xt[:], xs[n])
                ps = []
                for m in range(MT):
                    pm = pspool.tile([P, NT], mybir.dt.float32,
                                     name=f"ps{m}", tag="ps")
                    ps.append(pm)
                for k in range(KT):
                    for m in range(MT):
                        nc.tensor.matmul(
                            ps[m][:],
                            wap(sel, k, m),
                            xt[:, k * NT:(k + 1) * NT],
                            start=(k == 0),
                            stop=(k == KT - 1),
                        )
                osup = opool.tile([P, MT * NT], mybir.dt.float32,
                                  name="osup", tag="osup")
                for m in range(MT):
                    nc.scalar.activation(
                        osup[:, m * NT:(m + 1) * NT], ps[m][:],
                        mybir.ActivationFunctionType.Relu,
                        bias=btile[:, sel * MT + m:sel * MT + m + 1],
                    )
                nc.sync.dma_start(ys[n], osup[:])
    nc.compile()
    return nc


def _get_program(CA: int, CB: int) -> bass.Bass:
    key = (CA, CB)
    if key not in _PROGRAM_CACHE:
        _PROGRAM_CACHE[key] = _build_program(CA, CB)
    return _PROGRAM_CACHE[key]


def _pad(n: int) -> int:
    return int(max(NT, math.ceil(n / NT) * NT))


def _route(x, indices):
    """Host-side routing: stable sort by expert, hot/cold pairing, padding."""
    idx = np.asarray(indices).reshape(-1).astype(np.int64)
    order = np.argsort(idx, kind="stable")
    counts = np.bincount(idx, minlength=E)
    starts = np.concatenate([[0], np.cumsum(counts)])
    tok = {e: order[starts[e]:starts[e + 1]] for e in range(E)}

    by_count = np.argsort(-counts, kind="stable")
    pairs = [(int(by_count[i]), int(by_count[E - 1 - i])) for i in range(E // 2)]
    CA = _pad(max(int(counts[a]) for a, _ in pairs))
    CB = _pad(max(int(counts[b]) for _, b in pairs))
    return order, counts, tok, pairs, CA, CB


def _swizzle_x(x, tok_a, tok_b, CA, CB):
    """[C2, D] padded token matrix -> [NCH, P, KT*NT] per-chunk-contiguous."""
    C2 = CA + CB
    xp = np.zeros((C2, D), dtype=np.float32)
    if len(tok_a):
        xp[:len(tok_a)] = x[tok_a]
    if len(tok_b):
        xp[CA:CA + len(tok_b)] = x[tok_b]
    # xs[n, p, k, t] = xp[n*NT + t, k*P + p]
    return np.ascontiguousarray(
        xp.reshape(C2 // NT, NT, KT, P).transpose(0, 3, 2, 1)
    ).reshape(C2 // NT, P, KT * NT)


def _swizzle_w(We, half):
    """W[e] [D, H] -> [P, KT*HH] for one H-half: Wc[p, k*HH+h] = W[k*P+p, hs+h]."""
    hs = slice(half * HH, (half + 1) * HH)
    return np.ascontiguousarray(
        We[:, hs].reshape(KT, P, HH).transpose(1, 0, 2)).reshape(P, KT * HH)


def _build_in_maps(x, W, b, counts, tok, pairs, CA, CB):
    x = np.asarray(x, dtype=np.float32)
    W = np.asarray(W, dtype=np.float32)
    b = np.asarray(b, dtype=np.float32)
    in_maps = []
    for (ea, eb) in pairs:
        xs_pair = _swizzle_x(x, tok[ea], tok[eb], CA, CB)
        for half in range(2):
            hs = slice(half * HH, (half + 1) * HH)
            bc = np.stack([b[ea][hs].reshape(MT, P),
                           b[eb][hs].reshape(MT, P)])  # [2, MT, P]
            in_maps.append({
                "xs": xs_pair,
                "Wc": np.stack([_swizzle_w(W[ea], half),
                                _swizzle_w(W[eb], half)]),
                "bc": np.ascontiguousarray(
                    bc.reshape(2 * MT, P).T),          # [P, 2*MT]
            })
    return in_maps


def _assemble(results, N, counts, pairs, CA, CB):
    out = np.empty((N, H), dtype=np.float32)
    starts = {}
    pos = 0
    for e in range(E):
        starts[e] = pos
        pos += int(counts[e])
    C2 = CA + CB
    for i, (ea, eb) in enumerate(pairs):
        ca, cb = int(counts[ea]), int(counts[eb])
        for half in range(2):
            ysw = results[2 * i + half]["ys"]  # [NCH, P, MT*NT]
            # y[tok n*NT+t, hs + m*P + p] = ysw[n, p, m*NT + t]
            y = ysw.reshape(C2 // NT, P, MT, NT).transpose(0, 3, 2, 1) \
                   .reshape(C2, HH)
            hs = slice(half * HH, (half + 1) * HH)
            if ca:
                out[starts[ea]:starts[ea] + ca, hs] = y[:ca]
            if cb:
                out[starts[eb]:starts[eb] + cb, hs] = y[CA:CA + cb]
    return out


def kernel(x, indices, W, b):
    x = np.asarray(x, dtype=np.float32)
    N = x.shape[0]
    order, counts, tok, pairs, CA, CB = _route(x, indices)
    nc = _get_program(CA, CB)
    in_maps = _build_in_maps(x, W, b, counts, tok, pairs, CA, CB)
    results = run_bass_kernel_spmd(nc, in_maps, list(range(E))).results
    return _assemble(results, N, counts, pairs, CA, CB)
